# revision 20
# baseline (speedup 1.0000x reference)
"""Trainium2 Bass kernel for causal multi-head attention.

Problem shapes (hardcoded):
  x: [B=2, S=2048, D_MODEL=768] f32
  W_Q/W_K/W_V: [H=12, 768, 64], W_O: [12, 64, 768], b_*: per-head biases
  out: [2, 2048, 768] f32

Sharding: 8 cores; core c owns batch b = c // 4 and heads [3*(c%4), 3*(c%4)+3).
Each core computes a partial output over its 3 heads; the host sums the 4
partials per batch (the "all-reduce" of the output projection).

Single software-pipelined schedule (vs the phase-serial v1): all 3 heads'
scores/exp/z' advance together per (qt, kb-pair) group, with V-projection
chunks and output-projection tiles interleaved into the PE stream so the PE
never idles (keeps the HAM clock at 8/8) and the ACT engine's exp stream
hides under PE work.  Normalize is fully on-chip: DVE reciprocal_approx_fast
on the PSUM denominator row, gpsimd partition_broadcast, DVE multiply --
no DRAM round-trips.  h1's z' lands on PSUM partitions 63:128 (ones-column
first in its V pack) so the DVE multiply writes zT01[64:128] without a
partition-shift DMA.  Output tiles are written bf16 (host sums in f32).
"""

import sys
import types

for _p in ("/opt/trn_rl_repo",):
    if _p not in sys.path:
        sys.path.insert(0, _p)

import numpy as np
import ml_dtypes

BF16 = ml_dtypes.bfloat16

B, S, D_MODEL, N_HEADS, D_HEAD = 2, 2048, 768, 12, 64
N_CORES = 8
HEADS_PER_CORE = 3
SCALE = 1.0 / 8.0  # 1/sqrt(d_head)

_CACHE = {}


def _ensure_ntff_hook():
    """Register the axon NTFF profile hook if the image lacks antenv.axon_hooks."""
    try:
        import antenv.axon_hooks  # noqa: F401
        return
    except ImportError:
        pass
    import antenv
    mod = types.ModuleType("antenv.axon_hooks")
    _h = [None]
    mod.set_axon_ntff_profile_hook = lambda h: _h.__setitem__(0, h)
    mod.get_axon_ntff_profile_hook = lambda: _h[0]
    sys.modules["antenv.axon_hooks"] = mod
    antenv.axon_hooks = mod
    try:
        from trn_agent_boot.trn_boot import _ntff_profile_via_ctypes
        hook = _ntff_profile_via_ctypes("/opt/axon/libaxon_pjrt.so")
        if hook is not None:
            mod.set_axon_ntff_profile_hook(hook)
    except Exception:
        pass


def build_bass():
    """Build and compile the per-core Bass program (same NEFF on all 8 cores)."""
    if "nc" in _CACHE:
        return _CACHE["nc"]

    import concourse.mybir as mybir
    import concourse.tile as tile
    from concourse import bacc
    from concourse.bass import ts
    from contextlib import ExitStack

    f32 = mybir.dt.float32
    bf16 = mybir.dt.bfloat16
    Exp = mybir.ActivationFunctionType.Exp
    Ident = mybir.ActivationFunctionType.Identity

    nc = bacc.Bacc("TRN2", target_bir_lowering=False, debug=False, num_devices=N_CORES)

    xt_d = nc.dram_tensor("xt", [768, 2048], bf16, kind="ExternalInput").ap()
    wqk_d = nc.dram_tensor("wqk", [768, 384], bf16, kind="ExternalInput").ap()
    wv_d = nc.dram_tensor("wv", [768, 195], bf16, kind="ExternalInput").ap()
    wo_d = nc.dram_tensor("wo", [193, 768], bf16, kind="ExternalInput").ap()
    bqk_d = nc.dram_tensor("bqk", [128, 3], f32, kind="ExternalInput").ap()
    mask_d = nc.dram_tensor("mask", [128, 128], bf16, kind="ExternalInput").ap()
    out_d = nc.dram_tensor("out", [2048, 768], bf16, kind="ExternalOutput").ap()

    with tile.TileContext(nc) as tc, ExitStack() as ctx:
        setup = ctx.enter_context(tc.tile_pool(name="setup", bufs=1))
        expp = ctx.enter_context(tc.tile_pool(name="expp", bufs=6))
        otp = ctx.enter_context(tc.tile_pool(name="otp", bufs=4))
        rnp = ctx.enter_context(tc.tile_pool(name="rnp", bufs=2))
        ps = ctx.enter_context(tc.tile_pool(name="ps", bufs=2, space="PSUM"))
        drp = ctx.enter_context(tc.tile_pool(name="drp", bufs=2, space="DRAM"))

        # ---- tiny memsets first (idle engines; all off the critical path) --
        wsrc = setup.tile([128, 512], bf16, tag="wsrc")
        nc.vector.memset(wsrc[:], 0.0)
        onesf = setup.tile([128, 64], f32, tag="onesf")
        nc.vector.memset(onesf[:], 1.0)

        # persistent activations / staging
        QT01 = setup.tile([128, 2048], bf16, tag="QT01")
        KT01 = setup.tile([128, 2048], bf16, tag="KT01")
        Q2T = setup.tile([64, 2048], bf16, tag="Q2T")
        K2T = setup.tile([64, 2048], bf16, tag="K2T")
        Q2Tb = setup.tile([128, 2048], bf16, tag="Q2Tb")  # rows 64:128 used
        K2Tb = setup.tile([128, 2048], bf16, tag="K2Tb")
        # V packs per 128-seq chunk: [V_h|1] per head -> ones cols 64/129/194
        Vsb = [setup.tile([128, 195], bf16, tag=f"v{sb}", name=f"v{sb}")
               for sb in range(16)]
        # normalized z, transposed, per q-tile; zT2q row 64 = ones (bias fold)
        zT01q = [setup.tile([128, 512], bf16, tag=f"zT01q{q}", name=f"zT01q{q}")
                 for q in range(4)]
        zT2q = [setup.tile([65, 512], bf16, tag=f"zT2q{q}", name=f"zT2q{q}")
                for q in range(4)]
        for q in range(4):
            nc.gpsimd.memset(zT2q[q][64:65, :], 1.0)

        # ---- input DMAs (SP queue, need-ordered) ----
        bqk = setup.tile([128, 3], f32, tag="bqk")
        nc.sync.dma_start(bqk[:], bqk_d[:, :])
        wqk = []
        for mc in range(6):
            t = setup.tile([128, 384], bf16, tag=f"wqk{mc}", name=f"wqk{mc}")
            nc.sync.dma_start(t[:], wqk_d[ts(mc, 128), :])
            wqk.append(t)
        # xt split into column halves for earlier first-tile availability
        xtH = [[], []]
        for mc in range(6):
            t = setup.tile([128, 1024], bf16, tag=f"xtA{mc}", name=f"xtA{mc}")
            nc.sync.dma_start(t[:], xt_d[ts(mc, 128), 0:1024])
            xtH[0].append(t)
        mask = setup.tile([128, 128], bf16, tag="mask")
        nc.sync.dma_start(mask[:], mask_d[:, :])
        wv = []
        for mc in range(6):
            t = setup.tile([128, 195], bf16, tag=f"wv{mc}", name=f"wv{mc}")
            nc.sync.dma_start(t[:], wv_d[ts(mc, 128), :])
            wv.append(t)
        for mc in range(6):
            t = setup.tile([128, 1024], bf16, tag=f"xtB{mc}", name=f"xtB{mc}")
            nc.sync.dma_start(t[:], xt_d[ts(mc, 128), 1024:2048])
            xtH[1].append(t)
        wo01 = setup.tile([128, 768], bf16, tag="wo01")
        nc.sync.dma_start(wo01[:], wo_d[0:128, :])
        wo2a = setup.tile([65, 768], bf16, tag="wo2a")
        nc.sync.dma_start(wo2a[:], wo_d[128:193, :])

        def xt_st(st):
            """[128,512] rhs slice of xt for seq tile st (0..3)."""
            return lambda mc: xtH[st // 2][mc][:, (st % 2) * 512:(st % 2) * 512 + 512]

        def xt_sb(sb):
            """[128,128] lhsT slice of xt for 128-seq chunk sb (0..15)."""
            h = sb // 8
            o = (sb % 8) * 128
            return lambda mc: xtH[h][mc][:, o:o + 128]

        # ---- PE warmup: flips the HAM clock gate to 8/8 (2.4 GHz) while
        # the input DMAs stream in ----
        wps = ps.tile([128, 1024], f32, tag="sc", name="warm")
        for _ in range(18):
            nc.tensor.matmul(wps[:, 0:512], lhsT=wsrc[:, 0:128], rhs=wsrc[:, :],
                             start=True, stop=True)
        wout = setup.tile([1, 4], f32, tag="wout")
        nc.vector.tensor_copy(wout[:], wps[0:1, 0:4])

        # ================= QKV projections =================
        # One st-column tile at a time so they interleave into the attention
        # pipeline: scores for q-tile qt only need K columns <= 512*(qt+1).
        # pi=0: [Q0|Q1] -> QT01; pi=1: [K0|K1] -> KT01; pi=2: [K2|Q2]
        def qk_proj(pi, st, tag):
            c0 = 128 * pi if pi < 2 else 256
            p = ps.tile([128, 512], f32, tag=tag, name=f"qk{pi}_{st}", bufs=1)
            for mc in range(6):
                nc.tensor.matmul(
                    p[:, :],
                    lhsT=wqk[mc][:, c0:c0 + 128],
                    rhs=xt_st(st)(mc),
                    start=(mc == 0),
                    stop=(mc == 5),
                )
            if pi == 0:
                nc.vector.tensor_scalar_add(
                    QT01[:, ts(st, 512)], p[:, :], bqk[:, 0:1])
            elif pi == 1:
                nc.vector.tensor_scalar_add(
                    KT01[:, ts(st, 512)], p[:, :], bqk[:, 1:2])
            else:
                # rows 0:64 = K2, rows 64:128 = Q2
                nc.vector.tensor_scalar_add(
                    K2T[:, ts(st, 512)], p[0:64, :], bqk[0:64, 2:3])
                nc.vector.tensor_scalar_add(
                    Q2Tb[64:128, ts(st, 512)], p[64:128, :], bqk[64:128, 2:3])
                # partition shifts for head-2 kb-parity pairing
                nc.sync.dma_start(Q2T[:, ts(st, 512)], Q2Tb[64:128, ts(st, 512)])
                nc.sync.dma_start(K2Tb[64:128, ts(st, 512)], K2T[:, ts(st, 512)])

        def v_proj(sb, tag="pv"):
            p = ps.tile([128, 195], f32, tag=tag, name=f"vp{sb}", bufs=1)
            for mc in range(6):
                nc.tensor.matmul(
                    p[:, :],
                    lhsT=xt_sb(sb)(mc),
                    rhs=wv[mc][:, :],
                    start=(mc == 0),
                    stop=(mc == 5),
                )
            nc.vector.tensor_copy(Vsb[sb][:, :], p[:, :])
            for h in range(3):
                nc.gpsimd.memset(Vsb[sb][:, h * 65 + 64:h * 65 + 65], 1.0)

        # front: only the st=0 column of Q/K and V chunks 0-3 (enough to
        # start qt=0); everything else streams in as deferred extras.
        qk_proj(0, 0, "t0")
        qk_proj(1, 0, "t1")
        qk_proj(2, 0, "t2")
        v_proj(0, "pv")
        v_proj(1, "t0")
        v_proj(2, "t1")
        v_proj(3, "t2")

        # ================= attention + outproj pipeline =================
        def normalize(qt, h, t, pe_bcast=False):
            """zT[h] cols qt*512.. = z / den; t = raw PSUM tile (z rows 0:64,
            den row 64).  First stage z'+den to SBUF with one DVE copy so the
            PSUM bank frees immediately (the WAR on it gates qt+1's z').
            reciprocal_approx_fast must run at partition base 0 (offset APs
            silently break it), so it covers rows 0:65; row 64 is 1/den.
            Broadcast 1/den across partitions via DRAM round-trip, or via a
            PE ones-matmul when the PE is idle (tail).  h1 lands on
            zT01[64:128], unreachable for DVE from rows 0:64 -- DMA shift."""
            zs = rnp.tile([65, 512], f32, tag=f"zs{h}", name=f"zs{h}_{qt}")
            nc.vector.tensor_copy(zs[:], t[0:65, :])
            r = rnp.tile([128, 512], f32, tag=f"rn{h}", name=f"rn{h}_{qt}")
            nc.vector.reciprocal_approx_fast(r[0:65, :], zs[0:65, :])
            if pe_bcast:
                rb = ps.tile([128, 512], f32, tag="pv", name=f"rb{h}_{qt}", bufs=1)
                nc.tensor.matmul(rb[0:64, :], lhsT=onesf[64:65, :],
                                 rhs=r[64:65, :], start=True, stop=True)
                nc.vector.tensor_copy(r[0:64, :], rb[0:64, :])
            else:
                dsc = drp.tile([1, 512], f32, tag=f"dsc{h}", name=f"dsc{h}_{qt}")
                nc.sync.dma_start(dsc[:], r[64:65, :])
                nc.sync.dma_start(r[0:64, :], dsc[0:1, :].broadcast_to((64, 512)))
            if h == 1:
                zsh = rnp.tile([64, 512], bf16, tag="zsh", name=f"zsh{qt}")
                nc.vector.tensor_mul(zsh[:], zs[0:64, :], r[0:64, :])
                nc.sync.dma_start(zT01q[qt][64:128, :], zsh[:])
            else:
                dst = zT01q[qt][0:64, :] if h == 0 else zT2q[qt][0:64, :]
                nc.vector.tensor_mul(dst, zs[0:64, :], r[0:64, :])

        def out_proj(st, on_act=False):
            qt = st // 4
            i = st % 4
            p = ps.tile([128, 1024], f32, tag="sc", name=f"op{st}")
            for n0, nw in ((0, 512), (512, 256)):
                nc.tensor.matmul(
                    p[:, n0:n0 + nw],
                    lhsT=zT01q[qt][:, ts(i, 128)],
                    rhs=wo01[:, n0:n0 + nw],
                    start=True, stop=False,
                    skip_group_check=True,
                )
                nc.tensor.matmul(
                    p[:, n0:n0 + nw],
                    lhsT=zT2q[qt][0:65, ts(i, 128)],
                    rhs=wo2a[0:65, n0:n0 + nw],
                    start=False, stop=True,
                    skip_group_check=True,
                )
            ot = otp.tile([128, 768], bf16, tag="ot", name=f"ot{st}")
            if on_act:
                nc.scalar.copy(ot[:], p[:, 0:768])
            else:
                nc.vector.tensor_copy(ot[:], p[:, 0:768])
            nc.sync.dma_start(out_d[ts(st, 128), :], ot[:])

        def run_extra(e):
            if e[0] == "p":
                qk_proj(e[1], e[2], "pv")
            elif e[0] == "v":
                v_proj(e[1], "pv")
            else:
                out_proj(e[1])

        VCOLS = {0: (0, 65), 1: (65, 130), 2: (130, 195)}

        for qt in range(4):
            nkb = 4 * qt + 4
            t0 = ps.tile([128, 512], f32, tag="t0", name=f"zp0_{qt}", bufs=1)
            t1 = ps.tile([128, 512], f32, tag="t1", name=f"zp1_{qt}", bufs=1)
            t2 = ps.tile([128, 512], f32, tag="t2", name=f"zp2_{qt}", bufs=1)
            zp = {0: t0[0:65, :], 1: t1[0:65, :], 2: t2[0:65, :]}
            # deferred PE work interleaved into this qt's groups: next st
            # column of the projections, V chunks for qt+1, outproj of qt-1
            extras = {
                0: [("p", 2, 1), ("p", 0, 1), ("p", 1, 1),
                    ("v", 4), ("v", 5), ("v", 6), ("v", 7)],
                1: [("p", 2, 2), ("p", 0, 2), ("p", 1, 2),
                    ("v", 8), ("v", 9)],
                2: [("p", 2, 3), ("p", 0, 3), ("p", 1, 3), ("v", 10), ("v", 11),
                    ("o", 0), ("o", 1), ("o", 2), ("o", 3)],
                3: [("v", 12), ("v", 13), ("v", 14), ("v", 15),
                    ("o", 4), ("o", 5), ("o", 6), ("o", 7),
                    ("o", 8), ("o", 9), ("o", 10), ("o", 11)],
            }[qt]
            ngroups = nkb // 2
            pops = -(-len(extras) // ngroups)  # ceil: spread across groups

            for g in range(nkb // 2):
                kbs = (2 * g, 2 * g + 1)
                qc0s = [max(0, 128 * (kb - 4 * qt)) for kb in kbs]
                diag = kbs[0] >= 4 * qt
                sc = {}
                for h in range(3):
                    sc[h] = ps.tile([128, 1024], f32, tag="sc", name=f"sc{h}")
                # scores: h0/h1 row-group paired; h2 paired across kb parity
                for j, kb in enumerate(kbs):
                    q0 = qt * 512 + qc0s[j]
                    q1 = (qt + 1) * 512
                    nc.tensor.matmul(
                        sc[0][:, 512 * j + qc0s[j]:512 * (j + 1)],
                        lhsT=KT01[0:64, ts(kb, 128)], rhs=QT01[0:64, q0:q1],
                        start=True, stop=True)
                    nc.tensor.matmul(
                        sc[1][:, 512 * j + qc0s[j]:512 * (j + 1)],
                        lhsT=KT01[64:128, ts(kb, 128)], rhs=QT01[64:128, q0:q1],
                        start=True, stop=True)
                for j, kb in enumerate(kbs):
                    q0 = qt * 512 + qc0s[j]
                    q1 = (qt + 1) * 512
                    if j == 0:
                        nc.tensor.matmul(
                            sc[2][:, qc0s[0]:512],
                            lhsT=K2T[:, ts(kb, 128)], rhs=Q2T[:, q0:q1],
                            start=True, stop=True)
                    else:
                        nc.tensor.matmul(
                            sc[2][:, 512 + qc0s[1]:1024],
                            lhsT=K2Tb[64:128, ts(kb, 128)],
                            rhs=Q2Tb[64:128, q0:q1],
                            start=True, stop=True)
                # exp (ACT only ever runs Exp: single table load)
                ex = {}
                for h in range(3):
                    e = expp.tile([128, 1024], bf16, tag="ex", name=f"ex{h}")
                    if not diag:
                        nc.scalar.activation(e[:], sc[h][:], Exp, scale=SCALE)
                    else:
                        for j in range(2):
                            s0 = 512 * j + qc0s[j]
                            nc.scalar.activation(
                                e[:, s0:512 * (j + 1)],
                                sc[h][:, s0:512 * (j + 1)],
                                Exp, scale=SCALE)
                            nc.gpsimd.tensor_mul(
                                e[:, s0:s0 + 128], e[:, s0:s0 + 128], mask[:])
                    ex[h] = e
                # z' accumulation (denominator rides the ones column)
                for j, kb in enumerate(kbs):
                    for h in range(3):
                        c0, c1 = VCOLS[h]
                        nc.tensor.matmul(
                            zp[h][:, qc0s[j]:512],
                            lhsT=Vsb[kb][:, c0:c1],
                            rhs=ex[h][:, 512 * j + qc0s[j]:512 * (j + 1)],
                            start=(kb == 0),
                            stop=(kb == nkb - 1),
                            skip_group_check=True,
                        )
                # keep PE fed: spread deferred work across the groups
                for _ in range(pops):
                    if extras:
                        run_extra(extras.pop(0))

            for e in extras:
                run_extra(e)
            for h, t in ((0, t0), (1, t1), (2, t2)):
                normalize(qt, h, t, pe_bcast=(qt == 3))

        # tail: last q-tile's output projection (copies split ACT/DVE)
        for st in range(12, 16):
            out_proj(st, on_act=(st % 2 == 0))

    nc.compile()
    _CACHE["nc"] = nc
    return nc


def _prep_core_inputs(c, x, W_Q, W_K, W_V, b_Q, b_K, b_V, W_O, b_O):
    b = c // 4
    h0 = HEADS_PER_CORE * (c % 4)
    hs = [h0, h0 + 1, h0 + 2]

    xt = x[b].T.astype(BF16)  # [768, 2048]

    # [Q0|Q1|K0|K1|K2|Q2] (pi=2 pass computes K2 rows 0:64, Q2 rows 64:128)
    wqk = np.concatenate(
        [W_Q[hs[0]], W_Q[hs[1]], W_K[hs[0]], W_K[hs[1]], W_K[hs[2]], W_Q[hs[2]]],
        axis=1,
    )  # [768, 384]
    bqk = np.stack([
        np.concatenate([b_Q[hs[0]], b_Q[hs[1]]]),
        np.concatenate([b_K[hs[0]], b_K[hs[1]]]),
        np.concatenate([b_K[hs[2]], b_Q[hs[2]]]),
    ], axis=1).astype(np.float32)

    # V pack columns [V0|g|V1|g|V2|g]; gap cols become the ones columns
    wv = np.zeros((768, 195), np.float32)
    wv[:, 0:64] = W_V[hs[0]]
    wv[:, 65:129] = W_V[hs[1]]
    wv[:, 130:194] = W_V[hs[2]]

    # b_O added once per batch group (4 cores sum); each core's own b_V
    # contribution is exact because attention rows sum to 1.
    bo_eff = b_O / 4.0 + sum(b_V[h] @ W_O[h] for h in hs)
    wo = np.concatenate(
        [W_O[hs[0]], W_O[hs[1]], W_O[hs[2]], bo_eff[None, :]], axis=0
    )  # [193, 768]

    kr = np.arange(128)[:, None]
    cc = np.arange(128)[None, :]
    mask = (cc >= kr)  # [128, 128] causal triangle (keep q >= k)

    return {
        "xt": xt,
        "mask": mask.astype(BF16),
        "bqk": bqk,
        "wqk": wqk.astype(BF16),
        "wv": wv.astype(BF16),
        "wo": wo.astype(BF16),
    }


def run_sharded(inputs, trace=False, trace_cores=None):
    """Run the SPMD kernel; returns (out [2,2048,768] f32, BassKernelResults)."""
    _ensure_ntff_hook()
    from concourse.bass_utils import run_bass_kernel_spmd

    nc = build_bass()
    in_maps = [
        _prep_core_inputs(c, inputs["normalized_resid_pre"], inputs["W_Q"],
                          inputs["W_K"], inputs["W_V"], inputs["b_Q"], inputs["b_K"],
                          inputs["b_V"], inputs["W_O"], inputs["b_O"])
        for c in range(N_CORES)
    ]
    kwargs = {}
    if trace:
        kwargs["trace"] = True
        kwargs["trace_cores"] = trace_cores if trace_cores is not None else [0]
    res = run_bass_kernel_spmd(nc, in_maps, core_ids=list(range(N_CORES)), **kwargs)

    out = np.zeros((B, S, D_MODEL), np.float32)
    for c in range(N_CORES):
        out[c // 4] += np.asarray(res.results[c]["out"], dtype=np.float32)
    return out, res


def kernel(normalized_resid_pre, W_Q, W_K, W_V, b_Q, b_K, b_V, W_O, b_O):
    inputs = dict(normalized_resid_pre=np.asarray(normalized_resid_pre, np.float32),
                  W_Q=np.asarray(W_Q, np.float32), W_K=np.asarray(W_K, np.float32),
                  W_V=np.asarray(W_V, np.float32), b_Q=np.asarray(b_Q, np.float32),
                  b_K=np.asarray(b_K, np.float32), b_V=np.asarray(b_V, np.float32),
                  W_O=np.asarray(W_O, np.float32), b_O=np.asarray(b_O, np.float32))
    out, _ = run_sharded(inputs, trace=False)
    return out
